# revision 5
# baseline (speedup 1.0000x reference)
"""DeepseekMoE block-quantized MoE kernel for 8 Trainium2 NeuronCores.

Strategy (expert-parallel with host-side dispatch):
  - The routing table (selected_experts) is known on the host before launch,
    so the all-to-all "dispatch" is done on the host: for each expert e we
    gather the unique tokens routed to it (dedup across the top-k slots),
    transpose to [H, n_e], and pad to a common capacity C.
  - Experts are sharded 2-per-core across the 8 cores.  Each core runs a
    dense 3-matmul MLP (gate/up -> silu*up -> down) for its 2 experts in
    x^T / act^T layout so no on-device transposes are needed.
  - Block-dequantization (w * repeat(s, 128)) is folded into the host-side
    weight preparation, which also emits slab-contiguous weight layouts so
    every weight DMA is a pure linear copy (4KB per partition line).
  - All matmul operands are bf16: same 1 col/cycle PE rate as fp32r, but
    half the HBM traffic (faster pipeline fill, no DMA-induced PE stalls)
    and FWL-eligible LDWEIGHTS (fp32 operands block fast weight load).
  - The host scatters the per-expert outputs back to [T, K, H].
"""

import math

import numpy as np

T = 4096
TOPK = 6
E = 16
H = 2048
I = 1408
BS = 128           # quant block size
HT = H // 128      # 16 h-tiles
IT = I // 128      # 11 i-tiles
NCORES = 8
# Single-pass SBUF budget bound: (HT + IT) * 2 * W bytes of x+act per
# partition plus ~40KB of weight/output staging must fit in ~208KB.
MAX_W = 2368

_BUILT = {}
LAST_RESULTS = None  # stashed BassKernelResults for external harnesses


def _chunk_plan(width, small_first=False):
    """Split `width` columns into PSUM-bank-sized chunks (<=512), each >=256
    when width allows (small free dims pay LDWEIGHTS/dispatch overhead).
    With small_first, carve a 256-col leading chunk so the first matmul
    group's input DMA is small (faster pipeline fill at kernel start)."""
    if width <= 512:
        return [(0, width)]
    if small_first and width > 768:
        return [(0, 256)] + [(256 + o, w) for o, w in _chunk_plan(width - 256)]
    n = -(-width // 512)
    # 8-aligned chunk widths
    base = (width // n) // 8 * 8
    rem8 = (width - n * base) // 8
    out, off = [], 0
    for j in range(n):
        w = base + (8 if j < rem8 else 0)
        if j == n - 1:
            w = width - off
        out.append((off, w))
        off += w
    return out


def _build(jobs, CT):
    """Build the SPMD Bass program.  `jobs` is a tuple of
    (slot, col_offset, width): each job runs one expert slot's MLP over a
    window of `width` token columns; CT is the column capacity of xt/yt."""
    import concourse.bacc as bacc
    import concourse.mybir as mybir
    from concourse.bass import ts
    from concourse.tile import TileContext

    f32 = mybir.dt.float32
    bf16 = mybir.dt.bfloat16
    AF = mybir.ActivationFunctionType
    import os as _os

    act_fn = (
        AF.Sigmoid if _os.environ.get("KERNEL_SIM_SIGMOID") else AF.Silu
    )  # CoreSim lacks Silu; HW path always uses Silu

    nc = bacc.Bacc()
    xt = nc.declare_dram_parameter("xt", [2, HT, 128, CT], bf16, isOutput=False)
    # slab-contiguous weights: w0t/w1t slab i = [128, H]; w2t slab h = [128, I]
    w0t = nc.declare_dram_parameter("w0t", [2, IT, 128, H], bf16, isOutput=False)
    w1t = nc.declare_dram_parameter("w1t", [2, IT, 128, H], bf16, isOutput=False)
    w2t = nc.declare_dram_parameter("w2t", [2, HT, 128, I], bf16, isOutput=False)
    yt = nc.declare_dram_parameter("yt", [2, HT, 128, CT], bf16, isOutput=True)

    with TileContext(nc) as tc:
        with (
            tc.tile_pool(name="xp", bufs=1) as xp,
            tc.tile_pool(name="ap", bufs=1) as apool,
            tc.tile_pool(name="wp", bufs=2) as wp,
            tc.tile_pool(name="yp", bufs=4) as yp,
            tc.tile_pool(name="ps", bufs=3, space="PSUM") as ps,
        ):
            def load_w01_slab(which, src, s, i):
                slab = wp.tile([128, H], bf16, tag=which, name=None, bufs=3)
                nc.sync.dma_start(out=slab, in_=src[s, i])
                return slab

            for jn, (s, co, W) in enumerate(jobs):
                    chunks = _chunk_plan(W, small_first=(jn == 0))
                    xs = [
                        xp.tile([128, W], bf16, tag=f"x{h}", name=f"x{h}_{jn}")
                        for h in range(HT)
                    ]
                    # Bandwidth-priority emission: weight slabs for i=0..2 on
                    # the sync queue, x chunks (c-major) on the gpsimd queue so
                    # neither head-of-line blocks the other.  The PE can start
                    # as soon as the i=0 gate slab and x chunk 0 land.
                    slab_q = {0: [load_w01_slab("w0", w0t, s, 0)]}
                    for ci, (c0, cw) in enumerate(chunks):
                        for h in range(HT):
                            nc.gpsimd.dma_start(
                                out=xs[h][:, c0 : c0 + cw],
                                in_=xt[s, h, :, co + c0 : co + c0 + cw],
                            )
                        if ci == 0:
                            slab_q[0].append(load_w01_slab("w1", w1t, s, 0))
                        if ci <= 1 and ci < len(chunks) - 1 and ci + 1 < IT:
                            i_pre = ci + 1
                            slab_q[i_pre] = [
                                load_w01_slab("w0", w0t, s, i_pre),
                                load_w01_slab("w1", w1t, s, i_pre),
                            ]
                    acts = [
                        apool.tile([128, W], bf16, tag=f"a{i}", name=f"a{i}_{jn}")
                        for i in range(IT)
                    ]

                    # Phase A: gate/up projections + silu*up, per i-tile.
                    n_pre = max(slab_q) + 1
                    for i in range(IT):
                        w0s, w1s = slab_q.pop(i)
                        i_next = i + n_pre
                        if i_next < IT:
                            slab_q[i_next] = [
                                load_w01_slab("w0", w0t, s, i_next),
                                load_w01_slab("w1", w1t, s, i_next),
                            ]
                        for c0, cw in chunks:
                            g = ps.tile([128, 512], f32, tag="g")
                            for h in range(HT):
                                nc.tensor.matmul(
                                    g[:, :cw],
                                    w0s[:, ts(h, 128)],
                                    xs[h][:, c0 : c0 + cw],
                                    start=(h == 0),
                                    stop=(h == HT - 1),
                                )
                            u = ps.tile([128, 512], f32, tag="u")
                            for h in range(HT):
                                nc.tensor.matmul(
                                    u[:, :cw],
                                    w1s[:, ts(h, 128)],
                                    xs[h][:, c0 : c0 + cw],
                                    start=(h == 0),
                                    stop=(h == HT - 1),
                                )
                            a_sl = acts[i][:, c0 : c0 + cw]
                            nc.scalar.activation(a_sl, g[:, :cw], act_fn)
                            nc.vector.tensor_mul(a_sl, a_sl, u[:, :cw])

                    # Phase B: down projection, per h-tile.
                    for h in range(HT):
                        w2s = wp.tile([128, I], bf16, tag="w2", bufs=3)
                        nc.sync.dma_start(out=w2s, in_=w2t[s, h])
                        for c0, cw in chunks:
                            o = ps.tile([128, 512], f32, tag="o", bufs=2)
                            for i in range(IT):
                                nc.tensor.matmul(
                                    o[:, :cw],
                                    w2s[:, ts(i, 128)],
                                    acts[i][:, c0 : c0 + cw],
                                    start=(i == 0),
                                    stop=(i == IT - 1),
                                )
                            yc = yp.tile([128, 512], bf16, tag="y")
                            nc.vector.tensor_copy(yc[:, :cw], o[:, :cw])
                            nc.sync.dma_start(
                                out=yt[s, h, :, co + c0 : co + c0 + cw],
                                in_=yc[:, :cw],
                            )
    nc.finalize()
    return nc


def _get_built(jobs, CT):
    key = (tuple(jobs), CT)
    if key not in _BUILT:
        _BUILT[key] = _build(tuple(jobs), CT)
    return _BUILT[key]


def _dequant(w, s):
    """w: [E, O, Iin], s: [E, O, Iin//128] -> dequantized [E, O, Iin]."""
    e, o, iin = w.shape
    nb = -(-iin // BS)
    if nb * BS != iin:
        s_full = np.repeat(s, BS, axis=-1)[..., :iin]
        return w * s_full
    return (w.reshape(e, o, nb, BS) * s[..., None]).reshape(e, o, iin)


def _slabify(wd, bf16):
    """wd: [E, O, C] dequantized weights -> [E, O//128, 128, C] bf16 where
    slab o = [128 c-sub partitions, O-tile columns grouped by c-tile]:
    out[e, o, p, ct*128+j] = wd[e, o*128+j, ct*128+p]."""
    e, o, c = wd.shape
    ot, ct = o // 128, c // 128
    v = wd.astype(bf16).reshape(e, ot, 128, ct, 128)
    return v.transpose(0, 1, 4, 3, 2).reshape(e, ot, 128, c)


def kernel(**inputs):
    global LAST_RESULTS
    import ml_dtypes

    bf16 = ml_dtypes.bfloat16

    x = np.ascontiguousarray(np.asarray(inputs["x"], dtype=np.float32))
    sel = np.asarray(inputs["selected_experts"])
    w0 = np.asarray(inputs["w0"], dtype=np.float32)
    s0 = np.asarray(inputs["s0"], dtype=np.float32)
    w1 = np.asarray(inputs["w1"], dtype=np.float32)
    s1 = np.asarray(inputs["s1"], dtype=np.float32)
    w2 = np.asarray(inputs["w2"], dtype=np.float32)
    s2 = np.asarray(inputs["s2"], dtype=np.float32)

    t, k = sel.shape
    assert (t, k) == (T, TOPK) and x.shape == (T, H)

    # ---- host-side dispatch: unique tokens per expert ----
    pos = np.full((E, T), -1, dtype=np.int32)
    cols = []
    for e in range(E):
        toks = np.nonzero((sel == e).any(axis=1))[0]
        cols.append(toks)
        pos[e, toks] = np.arange(len(toks), dtype=np.int32)
    counts = np.array([len(c) for c in cols])

    # Assign experts to (core, slot): slot 0 holds the 8 largest experts,
    # slot 1 the 8 smallest, so each slot's padded width is only the max of
    # its own rank group.  expert_of[s][c] = expert on core c, slot s.
    order = np.argsort(-counts, kind="stable")
    expert_of = [list(order[:NCORES]), list(order[NCORES:])]

    def align8(v):
        return max(256, -(-v // 8) * 8)

    slot_w = [align8(int(counts[expert_of[s]].max())) for s in range(2)]

    if max(slot_w) <= MAX_W:
        jobs = tuple((s, 0, slot_w[s]) for s in range(2))
        CT = max(slot_w)
    else:
        # fallback: uniform width, multiple column windows per slot
        cmax = int(counts.max())
        passes = max(1, math.ceil(cmax / MAX_W))
        W = align8(math.ceil(cmax / passes))
        CT = W * passes
        jobs = tuple((s, cp * W, W) for s in range(2) for cp in range(passes))

    # ---- dequantize + slabify weights (host) ----
    # w0/w1: [E, I, H] -> slabs [E, IT, 128, H]; w2: [E, H, I] -> [E, HT, 128, I]
    w0s_all = _slabify(_dequant(w0, s0), bf16)
    w1s_all = _slabify(_dequant(w1, s1), bf16)
    w2s_all = _slabify(_dequant(w2, s2), bf16)

    x_bf = x.astype(bf16)
    in_maps = []
    for c in range(NCORES):
        pair = [expert_of[0][c], expert_of[1][c]]
        xt_c = np.zeros((2, H, CT), dtype=bf16)
        for s, e in enumerate(pair):
            n = len(cols[e])
            if n:
                xt_c[s, :, :n] = x_bf[cols[e]].T
        in_maps.append(
            {
                "xt": xt_c.reshape(2, HT, 128, CT),
                "w0t": np.ascontiguousarray(w0s_all[pair]),
                "w1t": np.ascontiguousarray(w1s_all[pair]),
                "w2t": np.ascontiguousarray(w2s_all[pair]),
            }
        )

    nc = _get_built(jobs, CT)
    from concourse.bass_utils import run_bass_kernel_spmd

    res = run_bass_kernel_spmd(nc, in_maps, list(range(NCORES)))
    LAST_RESULTS = res

    # Y[e] = [H, CT] for expert e
    Y = np.empty((E, H, CT), dtype=np.float32)
    for c in range(NCORES):
        yt_c = np.asarray(res.results[c]["yt"]).astype(np.float32).reshape(2, H, CT)
        Y[expert_of[0][c]] = yt_c[0]
        Y[expert_of[1][c]] = yt_c[1]

    # ---- scatter back to [T, K, H] ----
    e_flat = sel.reshape(-1).astype(np.int64)
    t_flat = np.repeat(np.arange(T, dtype=np.int64), TOPK)
    p_flat = pos[e_flat, t_flat]
    out = Y[e_flat, :, p_flat]  # [T*K, H]
    return np.ascontiguousarray(out.reshape(T, TOPK, H), dtype=np.float32)


# revision 7
# speedup vs baseline: 1.2127x; 1.2127x over previous
"""DeepseekMoE block-quantized MoE kernel for 8 Trainium2 NeuronCores.

Strategy (expert-parallel with host-side dispatch):
  - The routing table (selected_experts) is known on the host before launch,
    so the all-to-all "dispatch" is done on the host: for each expert e we
    gather the unique tokens routed to it (dedup across the top-k slots),
    transpose to [H, n_e], and pad to a common capacity C.
  - Experts are sharded 2-per-core across the 8 cores.  Each core runs a
    dense 3-matmul MLP (gate/up -> silu*up -> down) for its 2 experts in
    x^T / act^T layout so no on-device transposes are needed.
  - Block-dequantization (w * repeat(s, 128)) is folded into the host-side
    weight preparation, which also emits slab-contiguous weight layouts so
    every weight DMA is a pure linear copy (4KB per partition line).
  - All matmul operands are bf16: same 1 col/cycle PE rate as fp32r, but
    half the HBM traffic (faster pipeline fill, no DMA-induced PE stalls)
    and FWL-eligible LDWEIGHTS (fp32 operands block fast weight load).
  - The host scatters the per-expert outputs back to [T, K, H].
"""

import math

import numpy as np

T = 4096
TOPK = 6
E = 16
H = 2048
I = 1408
BS = 128           # quant block size
HT = H // 128      # 16 h-tiles
IT = I // 128      # 11 i-tiles
NCORES = 8
# Single-pass SBUF budget bound: (HT + IT) * 2 * W bytes of x+act per
# partition plus ~40KB of weight/output staging must fit in ~208KB.
MAX_W = 2368

_BUILT = {}
LAST_RESULTS = None  # stashed BassKernelResults for external harnesses


def _chunk_plan(width, small_first=False):
    """Split `width` columns into PSUM-bank-sized chunks (<=512), each >=256
    when width allows (small free dims pay LDWEIGHTS/dispatch overhead).
    With small_first, carve a 256-col leading chunk so the first matmul
    group's input DMA is small (faster pipeline fill at kernel start)."""
    if width <= 512:
        return [(0, width)]
    if small_first and width > 768:
        return [(0, 256)] + [(256 + o, w) for o, w in _chunk_plan(width - 256)]
    n = -(-width // 512)
    # 8-aligned chunk widths
    base = (width // n) // 8 * 8
    rem8 = (width - n * base) // 8
    out, off = [], 0
    for j in range(n):
        w = base + (8 if j < rem8 else 0)
        if j == n - 1:
            w = width - off
        out.append((off, w))
        off += w
    return out


def _build(jobs, CT):
    """Build the SPMD Bass program.  `jobs` is a tuple of
    (slot, col_offset, width): each job runs one expert slot's MLP over a
    window of `width` token columns; CT is the column capacity of xt/yt."""
    import concourse.bacc as bacc
    import concourse.mybir as mybir
    from concourse.bass import ts
    from concourse.tile import TileContext

    f32 = mybir.dt.float32
    bf16 = mybir.dt.bfloat16
    AF = mybir.ActivationFunctionType
    import os as _os

    act_fn = (
        AF.Sigmoid if _os.environ.get("KERNEL_SIM_SIGMOID") else AF.Silu
    )  # CoreSim lacks Silu; HW path always uses Silu

    nc = bacc.Bacc()
    xt = nc.declare_dram_parameter("xt", [2, HT, 128, CT], bf16, isOutput=False)
    # slab-contiguous weights: w0t/w1t slab i = [128, H]; w2t slab h = [128, I]
    w0t = nc.declare_dram_parameter("w0t", [2, IT, 128, H], bf16, isOutput=False)
    w1t = nc.declare_dram_parameter("w1t", [2, IT, 128, H], bf16, isOutput=False)
    w2t = nc.declare_dram_parameter("w2t", [2, HT, 128, I], bf16, isOutput=False)
    yt = nc.declare_dram_parameter("yt", [2, HT, 128, CT], bf16, isOutput=True)

    with TileContext(nc) as tc:
        with (
            tc.tile_pool(name="xp", bufs=1) as xp,
            tc.tile_pool(name="ap", bufs=1) as apool,
            tc.tile_pool(name="wp", bufs=2) as wp,
            tc.tile_pool(name="yp", bufs=4) as yp,
            tc.tile_pool(name="ps", bufs=3, space="PSUM") as ps,
        ):
            def load_w01_slab(which, src, s, i):
                slab = wp.tile([128, H], bf16, tag=which, name=None, bufs=3)
                nc.sync.dma_start(out=slab, in_=src[s, i])
                return slab

            for jn, (s, co, W) in enumerate(jobs):
                    chunks = _chunk_plan(W)
                    xs = [
                        xp.tile([128, W], bf16, tag=f"x{h}", name=f"x{h}_{jn}")
                        for h in range(HT)
                    ]
                    # Bandwidth-priority emission: weight slabs for i=0..2 on
                    # the sync queue, x h-tiles (full width, 2.8KB DMA lines)
                    # on the gpsimd queue so neither head-of-line blocks the
                    # other.  The PE starts once the i=0 gate slab and the
                    # first x h-tiles land.
                    slab_q = {0: [load_w01_slab("w0", w0t, s, 0)]}
                    for h in range(HT):
                        nc.gpsimd.dma_start(
                            out=xs[h], in_=xt[s, h, :, co : co + W]
                        )
                        if h == 0:
                            slab_q[0].append(load_w01_slab("w1", w1t, s, 0))
                        if h <= 1 and h + 1 < IT:
                            slab_q[h + 1] = [
                                load_w01_slab("w0", w0t, s, h + 1),
                                load_w01_slab("w1", w1t, s, h + 1),
                            ]
                    acts = [
                        apool.tile([128, W], bf16, tag=f"a{i}", name=f"a{i}_{jn}")
                        for i in range(IT)
                    ]

                    # Phase A: gate/up projections + silu*up, per i-tile.
                    n_pre = max(slab_q) + 1
                    for i in range(IT):
                        w0s, w1s = slab_q.pop(i)
                        i_next = i + n_pre
                        if i_next < IT:
                            slab_q[i_next] = [
                                load_w01_slab("w0", w0t, s, i_next),
                                load_w01_slab("w1", w1t, s, i_next),
                            ]
                        for c0, cw in chunks:
                            g = ps.tile([128, 512], f32, tag="g")
                            for h in range(HT):
                                nc.tensor.matmul(
                                    g[:, :cw],
                                    w0s[:, ts(h, 128)],
                                    xs[h][:, c0 : c0 + cw],
                                    start=(h == 0),
                                    stop=(h == HT - 1),
                                )
                            u = ps.tile([128, 512], f32, tag="u")
                            for h in range(HT):
                                nc.tensor.matmul(
                                    u[:, :cw],
                                    w1s[:, ts(h, 128)],
                                    xs[h][:, c0 : c0 + cw],
                                    start=(h == 0),
                                    stop=(h == HT - 1),
                                )
                            a_sl = acts[i][:, c0 : c0 + cw]
                            nc.scalar.activation(a_sl, g[:, :cw], act_fn)
                            nc.vector.tensor_mul(a_sl, a_sl, u[:, :cw])

                    # Phase B: down projection, per h-tile.  y is staged into
                    # a full-width SBUF row so the writeback is one DMA per h
                    # with 2.8KB partition lines.
                    for h in range(HT):
                        w2s = wp.tile([128, I], bf16, tag="w2", bufs=3)
                        nc.sync.dma_start(out=w2s, in_=w2t[s, h])
                        yc = yp.tile([128, W], bf16, tag="y", bufs=2)
                        for c0, cw in chunks:
                            o = ps.tile([128, 512], f32, tag="o", bufs=2)
                            for i in range(IT):
                                nc.tensor.matmul(
                                    o[:, :cw],
                                    w2s[:, ts(i, 128)],
                                    acts[i][:, c0 : c0 + cw],
                                    start=(i == 0),
                                    stop=(i == IT - 1),
                                )
                            nc.vector.tensor_copy(yc[:, c0 : c0 + cw], o[:, :cw])
                        nc.sync.dma_start(
                            out=yt[s, h, :, co : co + W], in_=yc
                        )
    nc.finalize()
    return nc


def _get_built(jobs, CT):
    key = (tuple(jobs), CT)
    if key not in _BUILT:
        _BUILT[key] = _build(tuple(jobs), CT)
    return _BUILT[key]


def _dequant(w, s):
    """w: [E, O, Iin], s: [E, O, Iin//128] -> dequantized [E, O, Iin]."""
    e, o, iin = w.shape
    nb = -(-iin // BS)
    if nb * BS != iin:
        s_full = np.repeat(s, BS, axis=-1)[..., :iin]
        return w * s_full
    return (w.reshape(e, o, nb, BS) * s[..., None]).reshape(e, o, iin)


def _slabify(wd, bf16):
    """wd: [E, O, C] dequantized weights -> [E, O//128, 128, C] bf16 where
    slab o = [128 c-sub partitions, O-tile columns grouped by c-tile]:
    out[e, o, p, ct*128+j] = wd[e, o*128+j, ct*128+p]."""
    e, o, c = wd.shape
    ot, ct = o // 128, c // 128
    v = wd.astype(bf16).reshape(e, ot, 128, ct, 128)
    return v.transpose(0, 1, 4, 3, 2).reshape(e, ot, 128, c)


def kernel(**inputs):
    global LAST_RESULTS
    import ml_dtypes

    bf16 = ml_dtypes.bfloat16

    x = np.ascontiguousarray(np.asarray(inputs["x"], dtype=np.float32))
    sel = np.asarray(inputs["selected_experts"])
    w0 = np.asarray(inputs["w0"], dtype=np.float32)
    s0 = np.asarray(inputs["s0"], dtype=np.float32)
    w1 = np.asarray(inputs["w1"], dtype=np.float32)
    s1 = np.asarray(inputs["s1"], dtype=np.float32)
    w2 = np.asarray(inputs["w2"], dtype=np.float32)
    s2 = np.asarray(inputs["s2"], dtype=np.float32)

    t, k = sel.shape
    assert (t, k) == (T, TOPK) and x.shape == (T, H)

    # ---- host-side dispatch: unique tokens per expert ----
    pos = np.full((E, T), -1, dtype=np.int32)
    cols = []
    for e in range(E):
        toks = np.nonzero((sel == e).any(axis=1))[0]
        cols.append(toks)
        pos[e, toks] = np.arange(len(toks), dtype=np.int32)
    counts = np.array([len(c) for c in cols])

    # Assign experts to (core, slot): slot 0 holds the 8 largest experts,
    # slot 1 the 8 smallest, so each slot's padded width is only the max of
    # its own rank group.  expert_of[s][c] = expert on core c, slot s.
    order = np.argsort(-counts, kind="stable")
    expert_of = [list(order[:NCORES]), list(order[NCORES:])]

    def align8(v):
        return max(256, -(-v // 8) * 8)

    slot_w = [align8(int(counts[expert_of[s]].max())) for s in range(2)]

    if max(slot_w) <= MAX_W:
        jobs = tuple((s, 0, slot_w[s]) for s in range(2))
        CT = max(slot_w)
    else:
        # fallback: uniform width, multiple column windows per slot
        cmax = int(counts.max())
        passes = max(1, math.ceil(cmax / MAX_W))
        W = align8(math.ceil(cmax / passes))
        CT = W * passes
        jobs = tuple((s, cp * W, W) for s in range(2) for cp in range(passes))

    # ---- dequantize + slabify weights (host) ----
    # w0/w1: [E, I, H] -> slabs [E, IT, 128, H]; w2: [E, H, I] -> [E, HT, 128, I]
    w0s_all = _slabify(_dequant(w0, s0), bf16)
    w1s_all = _slabify(_dequant(w1, s1), bf16)
    w2s_all = _slabify(_dequant(w2, s2), bf16)

    x_bf = x.astype(bf16)
    in_maps = []
    for c in range(NCORES):
        pair = [expert_of[0][c], expert_of[1][c]]
        xt_c = np.zeros((2, H, CT), dtype=bf16)
        for s, e in enumerate(pair):
            n = len(cols[e])
            if n:
                xt_c[s, :, :n] = x_bf[cols[e]].T
        in_maps.append(
            {
                "xt": xt_c.reshape(2, HT, 128, CT),
                "w0t": np.ascontiguousarray(w0s_all[pair]),
                "w1t": np.ascontiguousarray(w1s_all[pair]),
                "w2t": np.ascontiguousarray(w2s_all[pair]),
            }
        )

    nc = _get_built(jobs, CT)
    from concourse.bass_utils import run_bass_kernel_spmd

    res = run_bass_kernel_spmd(nc, in_maps, list(range(NCORES)))
    LAST_RESULTS = res

    # Y[e] = [H, CT] for expert e
    Y = np.empty((E, H, CT), dtype=np.float32)
    for c in range(NCORES):
        yt_c = np.asarray(res.results[c]["yt"]).astype(np.float32).reshape(2, H, CT)
        Y[expert_of[0][c]] = yt_c[0]
        Y[expert_of[1][c]] = yt_c[1]

    # ---- scatter back to [T, K, H] ----
    e_flat = sel.reshape(-1).astype(np.int64)
    t_flat = np.repeat(np.arange(T, dtype=np.int64), TOPK)
    p_flat = pos[e_flat, t_flat]
    out = Y[e_flat, :, p_flat]  # [T*K, H]
    return np.ascontiguousarray(out.reshape(T, TOPK, H), dtype=np.float32)


# revision 10
# speedup vs baseline: 1.2178x; 1.0042x over previous
"""DeepseekMoE block-quantized MoE kernel for 8 Trainium2 NeuronCores.

Strategy (expert-parallel with host-side dispatch):
  - The routing table (selected_experts) is known on the host before launch,
    so the all-to-all "dispatch" is done on the host: for each expert e we
    gather the unique tokens routed to it (dedup across the top-k slots),
    transpose to [H, n_e], and pad to a common capacity C.
  - Experts are sharded 2-per-core across the 8 cores.  Each core runs a
    dense 3-matmul MLP (gate/up -> silu*up -> down) for its 2 experts in
    x^T / act^T layout so no on-device transposes are needed.
  - Block-dequantization (w * repeat(s, 128)) is folded into the host-side
    weight preparation, which also emits slab-contiguous weight layouts so
    every weight DMA is a pure linear copy (4KB per partition line).
  - All matmul operands are bf16: same 1 col/cycle PE rate as fp32r, but
    half the HBM traffic (faster pipeline fill, no DMA-induced PE stalls)
    and FWL-eligible LDWEIGHTS (fp32 operands block fast weight load).
  - The host scatters the per-expert outputs back to [T, K, H].
"""

import math

import numpy as np

T = 4096
TOPK = 6
E = 16
H = 2048
I = 1408
BS = 128           # quant block size
HT = H // 128      # 16 h-tiles
IT = I // 128      # 11 i-tiles
NCORES = 8
# Single-pass width bound: at most 4 PSUM-bank chunks (the interleaved
# accumulation rings are 4 deep), and (HT + IT) * 2 * W bytes of x+act
# per partition plus ~40KB of staging must fit in ~208KB of SBUF.
MAX_W = 2040

_BUILT = {}
LAST_RESULTS = None  # stashed BassKernelResults for external harnesses


def _chunk_plan(width, small_first=False):
    """Split `width` columns into PSUM-bank-sized chunks (<=512), each >=256
    when width allows (small free dims pay LDWEIGHTS/dispatch overhead).
    With small_first, carve a 256-col leading chunk so the first matmul
    group's input DMA is small (faster pipeline fill at kernel start)."""
    if width <= 512:
        return [(0, width)]
    if small_first and width > 768:
        return [(0, 256)] + [(256 + o, w) for o, w in _chunk_plan(width - 256)]
    n = -(-width // 512)
    # 8-aligned chunk widths
    base = (width // n) // 8 * 8
    rem8 = (width - n * base) // 8
    out, off = [], 0
    for j in range(n):
        w = base + (8 if j < rem8 else 0)
        if j == n - 1:
            w = width - off
        out.append((off, w))
        off += w
    return out


def _build(jobs, CT):
    """Build the SPMD Bass program.  `jobs` is a tuple of
    (slot, col_offset, width): each job runs one expert slot's MLP over a
    window of `width` token columns; CT is the column capacity of xt/yt."""
    import concourse.bacc as bacc
    import concourse.mybir as mybir
    from concourse.bass import ts
    from concourse.tile import TileContext

    f32 = mybir.dt.float32
    bf16 = mybir.dt.bfloat16
    AF = mybir.ActivationFunctionType
    import os as _os

    act_fn = (
        AF.Sigmoid if _os.environ.get("KERNEL_SIM_SIGMOID") else AF.Silu
    )  # CoreSim lacks Silu; HW path always uses Silu

    nc = bacc.Bacc()
    xt = nc.declare_dram_parameter("xt", [2, HT, 128, CT], bf16, isOutput=False)
    # slab-contiguous weights: w0t/w1t slab i = [128, H]; w2t slab h = [128, I]
    w0t = nc.declare_dram_parameter("w0t", [2, IT, 128, H], bf16, isOutput=False)
    w1t = nc.declare_dram_parameter("w1t", [2, IT, 128, H], bf16, isOutput=False)
    w2t = nc.declare_dram_parameter("w2t", [2, HT, 128, I], bf16, isOutput=False)
    yt = nc.declare_dram_parameter("yt", [2, HT, 128, CT], bf16, isOutput=True)

    with TileContext(nc) as tc:
        with (
            tc.tile_pool(name="xp", bufs=1) as xp,
            tc.tile_pool(name="ap", bufs=1) as apool,
            tc.tile_pool(name="wp", bufs=2) as wp,
            tc.tile_pool(name="yp", bufs=4) as yp,
            tc.tile_pool(name="ps", bufs=3, space="PSUM") as ps,
        ):
            def load_w01_slab(which, src, s, i):
                slab = wp.tile([128, H], bf16, tag=which, name=None, bufs=3)
                nc.sync.dma_start(out=slab, in_=src[s, i])
                return slab

            for jn, (s, co, W) in enumerate(jobs):
                    chunks = _chunk_plan(W)
                    xs = [
                        xp.tile([128, W], bf16, tag=f"x{h}", name=f"x{h}_{jn}")
                        for h in range(HT)
                    ]
                    # Bandwidth-priority emission: weight slabs for i=0..2 on
                    # the sync queue, x h-tiles (full width, 2.8KB DMA lines)
                    # on the gpsimd queue so neither head-of-line blocks the
                    # other.  The PE starts once the i=0 gate slab and the
                    # first x h-tiles land.
                    slab_q = {0: [load_w01_slab("w0", w0t, s, 0)]}
                    for h in range(HT):
                        nc.gpsimd.dma_start(
                            out=xs[h], in_=xt[s, h, :, co : co + W]
                        )
                        if h == 0:
                            slab_q[0].append(load_w01_slab("w1", w1t, s, 0))
                        if h <= 1 and h + 1 < IT:
                            slab_q[h + 1] = [
                                load_w01_slab("w0", w0t, s, h + 1),
                                load_w01_slab("w1", w1t, s, h + 1),
                            ]
                    acts = [
                        apool.tile([128, W], bf16, tag=f"a{i}", name=f"a{i}_{jn}")
                        for i in range(IT)
                    ]

                    # Phase A: gate/up projections + silu*up, per i-tile.
                    # h-outer with chunk-interleaved PSUM accumulation: each
                    # xs[h] is consumed exactly once (streams at DMA delivery
                    # pace during the fill) and consecutive matmuls share the
                    # stationary weight tile.
                    n_pre = max(slab_q) + 1
                    for i in range(IT):
                        w0s, w1s = slab_q.pop(i)
                        i_next = i + n_pre
                        if i_next < IT:
                            slab_q[i_next] = [
                                load_w01_slab("w0", w0t, s, i_next),
                                load_w01_slab("w1", w1t, s, i_next),
                            ]
                        gb = [ps.tile([128, 512], f32, tag="g", bufs=4, name=f"g{jn}_{i}_{ci}") for ci in range(len(chunks))]
                        ub = [ps.tile([128, 512], f32, tag="u", bufs=4, name=f"u{jn}_{i}_{ci}") for ci in range(len(chunks))]
                        for h in range(HT):
                            for ci, (c0, cw) in enumerate(chunks):
                                nc.tensor.matmul(
                                    gb[ci][:, :cw],
                                    w0s[:, ts(h, 128)],
                                    xs[h][:, c0 : c0 + cw],
                                    start=(h == 0),
                                    stop=(h == HT - 1),
                                )
                            for ci, (c0, cw) in enumerate(chunks):
                                nc.tensor.matmul(
                                    ub[ci][:, :cw],
                                    w1s[:, ts(h, 128)],
                                    xs[h][:, c0 : c0 + cw],
                                    start=(h == 0),
                                    stop=(h == HT - 1),
                                )
                        for ci, (c0, cw) in enumerate(chunks):
                            a_sl = acts[i][:, c0 : c0 + cw]
                            nc.scalar.activation(a_sl, gb[ci][:, :cw], act_fn)
                            nc.vector.tensor_mul(a_sl, a_sl, ub[ci][:, :cw])

                    # Phase B: down projection, per h-tile, i-outer with the
                    # same chunk interleaving (PSUM banks shared with the "u"
                    # ring).  y is staged into a full-width SBUF row so the
                    # writeback is one DMA per h with 2.8KB partition lines.
                    for h in range(HT):
                        w2s = wp.tile([128, I], bf16, tag="w2", bufs=3)
                        nc.sync.dma_start(out=w2s, in_=w2t[s, h])
                        yc = yp.tile([128, W], bf16, tag="y", bufs=2)
                        ob = [ps.tile([128, 512], f32, tag="u", bufs=4, name=f"o{jn}_{h}_{ci}") for ci in range(len(chunks))]
                        for i in range(IT):
                            for ci, (c0, cw) in enumerate(chunks):
                                nc.tensor.matmul(
                                    ob[ci][:, :cw],
                                    w2s[:, ts(i, 128)],
                                    acts[i][:, c0 : c0 + cw],
                                    start=(i == 0),
                                    stop=(i == IT - 1),
                                )
                        for ci, (c0, cw) in enumerate(chunks):
                            nc.vector.tensor_copy(yc[:, c0 : c0 + cw], ob[ci][:, :cw])
                        nc.sync.dma_start(
                            out=yt[s, h, :, co : co + W], in_=yc
                        )
    nc.finalize()
    return nc


def _get_built(jobs, CT):
    key = (tuple(jobs), CT)
    if key not in _BUILT:
        _BUILT[key] = _build(tuple(jobs), CT)
    return _BUILT[key]


def _dequant(w, s):
    """w: [E, O, Iin], s: [E, O, Iin//128] -> dequantized [E, O, Iin]."""
    e, o, iin = w.shape
    nb = -(-iin // BS)
    if nb * BS != iin:
        s_full = np.repeat(s, BS, axis=-1)[..., :iin]
        return w * s_full
    return (w.reshape(e, o, nb, BS) * s[..., None]).reshape(e, o, iin)


def _slabify(wd, bf16):
    """wd: [E, O, C] dequantized weights -> [E, O//128, 128, C] bf16 where
    slab o = [128 c-sub partitions, O-tile columns grouped by c-tile]:
    out[e, o, p, ct*128+j] = wd[e, o*128+j, ct*128+p]."""
    e, o, c = wd.shape
    ot, ct = o // 128, c // 128
    v = wd.astype(bf16).reshape(e, ot, 128, ct, 128)
    return v.transpose(0, 1, 4, 3, 2).reshape(e, ot, 128, c)


def kernel(**inputs):
    global LAST_RESULTS
    import ml_dtypes

    bf16 = ml_dtypes.bfloat16

    x = np.ascontiguousarray(np.asarray(inputs["x"], dtype=np.float32))
    sel = np.asarray(inputs["selected_experts"])
    w0 = np.asarray(inputs["w0"], dtype=np.float32)
    s0 = np.asarray(inputs["s0"], dtype=np.float32)
    w1 = np.asarray(inputs["w1"], dtype=np.float32)
    s1 = np.asarray(inputs["s1"], dtype=np.float32)
    w2 = np.asarray(inputs["w2"], dtype=np.float32)
    s2 = np.asarray(inputs["s2"], dtype=np.float32)

    t, k = sel.shape
    assert (t, k) == (T, TOPK) and x.shape == (T, H)

    # ---- host-side dispatch: unique tokens per expert ----
    pos = np.full((E, T), -1, dtype=np.int32)
    cols = []
    for e in range(E):
        toks = np.nonzero((sel == e).any(axis=1))[0]
        cols.append(toks)
        pos[e, toks] = np.arange(len(toks), dtype=np.int32)
    counts = np.array([len(c) for c in cols])

    # Assign experts to (core, slot): slot 0 holds the 8 largest experts,
    # slot 1 the 8 smallest, so each slot's padded width is only the max of
    # its own rank group.  expert_of[s][c] = expert on core c, slot s.
    order = np.argsort(-counts, kind="stable")
    expert_of = [list(order[:NCORES]), list(order[NCORES:])]

    def align8(v):
        return max(256, -(-v // 8) * 8)

    slot_w = [align8(int(counts[expert_of[s]].max())) for s in range(2)]

    if max(slot_w) <= MAX_W:
        jobs = tuple((s, 0, slot_w[s]) for s in range(2))
        CT = max(slot_w)
    else:
        # fallback: uniform width, multiple column windows per slot
        cmax = int(counts.max())
        passes = max(1, math.ceil(cmax / MAX_W))
        W = align8(math.ceil(cmax / passes))
        CT = W * passes
        jobs = tuple((s, cp * W, W) for s in range(2) for cp in range(passes))

    # ---- dequantize + slabify weights (host) ----
    # w0/w1: [E, I, H] -> slabs [E, IT, 128, H]; w2: [E, H, I] -> [E, HT, 128, I]
    w0s_all = _slabify(_dequant(w0, s0), bf16)
    w1s_all = _slabify(_dequant(w1, s1), bf16)
    w2s_all = _slabify(_dequant(w2, s2), bf16)

    x_bf = x.astype(bf16)
    in_maps = []
    for c in range(NCORES):
        pair = [expert_of[0][c], expert_of[1][c]]
        xt_c = np.zeros((2, H, CT), dtype=bf16)
        for s, e in enumerate(pair):
            n = len(cols[e])
            if n:
                xt_c[s, :, :n] = x_bf[cols[e]].T
        in_maps.append(
            {
                "xt": xt_c.reshape(2, HT, 128, CT),
                "w0t": np.ascontiguousarray(w0s_all[pair]),
                "w1t": np.ascontiguousarray(w1s_all[pair]),
                "w2t": np.ascontiguousarray(w2s_all[pair]),
            }
        )

    nc = _get_built(jobs, CT)
    from concourse.bass_utils import run_bass_kernel_spmd

    res = run_bass_kernel_spmd(nc, in_maps, list(range(NCORES)))
    LAST_RESULTS = res

    # Y[e] = [H, CT] for expert e
    Y = np.empty((E, H, CT), dtype=np.float32)
    for c in range(NCORES):
        yt_c = np.asarray(res.results[c]["yt"]).astype(np.float32).reshape(2, H, CT)
        Y[expert_of[0][c]] = yt_c[0]
        Y[expert_of[1][c]] = yt_c[1]

    # ---- scatter back to [T, K, H] ----
    e_flat = sel.reshape(-1).astype(np.int64)
    t_flat = np.repeat(np.arange(T, dtype=np.int64), TOPK)
    p_flat = pos[e_flat, t_flat]
    out = Y[e_flat, :, p_flat]  # [T*K, H]
    return np.ascontiguousarray(out.reshape(T, TOPK, H), dtype=np.float32)
